# revision 49
# baseline (speedup 1.0000x reference)
"""Trainium2 Bass kernel for nn_CasparLayer (embedding -> GRU(reset_after) -> dense).

Problem shapes: B=128, T=256, VOCAB=41, EMB=512, HID=1024.

Strategy (per NeuronCore, SPMD x8):
  - The embedding lookup and the input projection are FUSED into the
    recurrent matmul: x_proj_t = onehot(x_t) @ (emb_table @ gru_kernel + b)
    rides as one extra K=41 chunk of the per-step contraction (one-hot rows
    sum to 1, so per-row-broadcast biases come along for free).
  - Keras masking (h = where(m, h_new, h)) is folded into the z-gate:
    z' = sigmoid(zr + (1-m)*30) == 1 for padded steps, so h carries over.
  - The PE 128x128 array is split into 4 column groups (tile_position=
    (0,32g)), each holding the same [K,32] stationary h-chunk and streaming
    its own quarter of the weight columns - 4 concurrent weight streams at
    M=32.
  - Keeping the PE *continuously* busy is critical: idle gaps drop it out of
    its fast clock state (measured 103ns vs 200ns per 4-group batch of
    N=256). So each step's h-independent matmuls (one-hot x-projections,
    biases) are pre-issued into the previous step's activation tail, gate
    outputs are computed in column halves so sigmoids/tanh/blend pipeline
    with the remaining matmuls, and the transposes/dense fill the rest.

Activations use the 'F-layout': a [32, 1024] quantity is stored as
[128, 256] with partition = 32*(h//256) + b, free = h % 256.

The harness contract: kernel(**inputs) takes full unsharded numpy inputs and
returns the full [128, 256, 41] float32 logits.
"""

import contextlib
import ctypes
import os
import sys
import types

sys.path.insert(0, "/opt/trn_rl_repo")

import numpy as np
import ml_dtypes

import bass_rust
import concourse.bass as bass
import concourse.tile as tile
from concourse import mybir
from concourse.alu_op_type import AluOpType

B = 128
T = 256
VOCAB = 41
EMB = 512
HID = 1024
H3 = 3 * HID
N_CORES = 8
BQ = 32  # batch quarter per core (4-way data parallel, x2 replicas)
Q = 4   # PE column groups
HH = 128  # half of a group's 256-column section (pipelining granule)
OH_WIN = 256  # one-hot SBUF window = all steps: slots written once, no WAR
DW = 8   # dense-head batching window (steps)
WARM = int(os.environ.get("BASS_GRU_WARM", "3"))  # tail warmer batches

F32 = mybir.dt.float32
BF16 = mybir.dt.bfloat16
AF = mybir.ActivationFunctionType


# ---------------------------------------------------------------------------
# Workaround: this walrus build accepts at most ONE sync wait per instruction;
# Tile attaches several. Hoist extras onto single-wait NOPs inserted before.
# ---------------------------------------------------------------------------
def _split_multiwaits(nc, max_waits: int = 1) -> int:
    n_split = 0
    for fn in nc.m.functions:
        for blk in fn.blocks:
            insts = blk.instructions
            i = 0
            while i < len(insts):
                ins = insts[i]
                si = ins.sync_info
                if si is not None and len(si.on_wait) > max_waits:
                    waits = list(si.on_wait)
                    keep = waits[-max_waits:]
                    hoist = waits[:-max_waits]
                    ins.sync_info = bass_rust.SyncInfo(
                        on_wait=keep, on_update=list(si.on_update)
                    )
                    for w in hoist:
                        nop = mybir.InstNoOp(
                            name=nc.get_next_instruction_name(),
                            sync_info=bass_rust.SyncInfo(on_wait=[w], on_update=[]),
                            bass_nofuse=True,
                            engine=ins.engine,
                            text_hint="wait_split",
                        )
                        nc.register_instruction(nop)
                        blk.instructions.insert(i, nop)
                        i += 1
                        n_split += 1
                i += 1
    return n_split


# ---------------------------------------------------------------------------
# Optional NTFF profiling under axon (the container's antenv stub lacks the
# hook registration module). Enabled via BASS_GRU_TRACE=1.
# ---------------------------------------------------------------------------
def _register_axon_profile_hook():
    so_path = "/opt/axon/libaxon_pjrt.so"
    if "antenv.axon_hooks" in sys.modules:
        return
    mod = types.ModuleType("antenv.axon_hooks")
    state = {"hook": None}
    mod.set_axon_ntff_profile_hook = lambda h: state.__setitem__("hook", h)
    mod.get_axon_ntff_profile_hook = lambda: state["hook"]
    sys.modules["antenv.axon_hooks"] = mod

    try:
        lib = ctypes.CDLL(so_path)
    except OSError:
        return
    if not hasattr(lib, "axon_start_nrt_profile"):
        return
    lib.axon_start_nrt_profile.argtypes = [
        ctypes.POINTER(ctypes.c_int64),
        ctypes.c_size_t,
    ]
    lib.axon_start_nrt_profile.restype = ctypes.c_int64
    lib.axon_stop_nrt_profile.argtypes = [ctypes.c_char_p]
    lib.axon_stop_nrt_profile.restype = ctypes.c_int64

    @contextlib.contextmanager
    def _hook_cm(output_dir, device_ids):
        import jax

        jax.devices()
        if device_ids:
            ids = (ctypes.c_int64 * len(device_ids))(*device_ids)
            rc = lib.axon_start_nrt_profile(ids, len(device_ids))
        else:
            rc = lib.axon_start_nrt_profile(None, 0)
        if rc != 0:
            raise RuntimeError(f"axon_start_nrt_profile rc={rc}")
        try:
            yield
        finally:
            n = lib.axon_stop_nrt_profile(str(output_dir).encode())
            print(f"ntff profile: {n} file(s) -> {output_dir}", file=sys.stderr)

    state["hook"] = _hook_cm

    import concourse.bass_utils as bu

    bu.upload_artifacts = lambda tmpdir: ""


# ---------------------------------------------------------------------------
# Kernel builder
# ---------------------------------------------------------------------------
def build_kernel(n_steps: int = T):
    nc = bass.Bass()

    wrec_d = nc.declare_dram_parameter("wrec", [HID, H3], BF16, isOutput=False)
    gzr_d = nc.declare_dram_parameter("gcat_zr", [VOCAB, 2 * HID], BF16, isOutput=False)
    grx_d = nc.declare_dram_parameter("gcat_rx", [VOCAB, 2 * HID], BF16, isOutput=False)
    oh_d = nc.declare_dram_parameter("onehot", [VOCAB, n_steps, BQ], BF16, isOutput=False)
    zb_d = nc.declare_dram_parameter("zbias", [128, n_steps], F32, isOutput=False)
    dw_d = nc.declare_dram_parameter("dw", [HID, VOCAB], BF16, isOutput=False)
    db_d = nc.declare_dram_parameter("db", [1, VOCAB], BF16, isOutput=False)
    n_dw = (n_steps + DW - 1) // DW
    out_d = nc.declare_dram_parameter(
        "logits", [n_dw, VOCAB, DW * BQ], F32, isOutput=True
    )

    KC = HID // 128  # 8 hidden-contraction chunks
    KD = KC // 2
    HQ = HID // Q    # 256 columns per group section

    with tile.TileContext(nc) as tc:
        with contextlib.ExitStack() as ctx:
            singles = ctx.enter_context(tc.tile_pool(name="singles", bufs=1))
            state = ctx.enter_context(tc.tile_pool(name="state", bufs=1))
            temps = ctx.enter_context(tc.tile_pool(name="temps", bufs=2))
            ps_zr = ctx.enter_context(tc.tile_pool(name="ps_zr", bufs=2, space="PSUM"))
            ps_rx = ctx.enter_context(tc.tile_pool(name="ps_rx", bufs=2, space="PSUM"))
            ps_dp = ctx.enter_context(tc.tile_pool(name="ps_dp", bufs=1, space="PSUM"))
            ps_wm = ctx.enter_context(tc.tile_pool(name="ps_wm", bufs=1, space="PSUM"))

            # --- weights / constants resident in SBUF ---
            # Weight columns are host-permuted per gate so PE group g owns the
            # strided h-columns {128u + 32g + cc}: F-layout partition=32g+b,
            # free=32u+cc. A DVE 32x32 block transpose of h then yields hT
            # chunks directly in SBUF (chunk u at free cols 32u:32u+32).
            w_all = singles.tile([128, KC, H3], BF16)
            nc.sync.dma_start(out=w_all, in_=wrec_d.rearrange("(c p) n -> p c n", p=128))
            # one-hot weights, merged per PE group g for single N=512 passes:
            # gcat_zr: [z_g 256 | r_g 256]; gcat_rx: [b1h_g 256 | xh_g 256]
            # (b1h rows replicated per vocab entry: one-hot rows sum to 1)
            gzr = singles.tile([VOCAB, 2 * HID], BF16)
            nc.sync.dma_start(out=gzr, in_=gzr_d[:])
            grx = singles.tile([VOCAB, 2 * HID], BF16)
            nc.sync.dma_start(out=grx, in_=grx_d[:])
            dw = singles.tile([128, KC, VOCAB], BF16)
            nc.sync.dma_start(out=dw, in_=dw_d.rearrange("(c p) n -> p c n", p=128))
            db = singles.tile([1, VOCAB], BF16)
            nc.sync.dma_start(out=db, in_=db_d[:])
            zb = singles.tile([128, n_steps], F32)
            nc.sync.dma_start(out=zb, in_=zb_d[:])
            ones = singles.tile([1, DW * BQ], BF16)
            nc.vector.memset(ones, 1.0)

            # one-hot window streamed via the sync DMA queue (logits go out on
            # the gpsimd queue so they never block these)
            win = min(OH_WIN, n_steps)
            pd = max(1, win // 2)
            ohw = singles.tile([VOCAB, win, BQ], BF16)
            for t in range(min(pd, n_steps)):
                nc.sync.dma_start(out=ohw[:, t % win, :], in_=oh_d[:, t, :])

            # --- GRU state: h in F-layout (ping-pong); hT in a DW-deep ring
            # of [128, 256] slots (chunk u at free cols 32u:32u+32); the ring
            # depth lets the dense head batch DW steps at once. ---
            h_st = [
                state.tile([128, HQ], BF16, tag=f"h{i}", name=f"h{i}") for i in range(2)
            ]
            hT_roll = state.tile([128, DW, HQ], BF16, tag="hTr", name="hTr")
            nc.vector.memset(h_st[0], 0.0)
            nc.vector.memset(hT_roll, 0.0)
            warm_ps = ps_wm.tile([128, 2 * HQ], F32, tag="warm", name="warm")

            def chunk(slot, c):
                return hT_roll[:, slot, 32 * c : 32 * c + 32]

            def preissue(t):
                """h-independent matmuls for step t: one-hot x-projections
                (embedding+input-proj fused, includes b0 and b1[z,r]) and the
                rh-gate b1h bias. These fill the PE during step t-1's
                activation tail and open every accumulation region.

                PSUM start=True zeroes the ENTIRE 2KB bank row for the
                written partitions, so exactly ONE start per bank per
                partition-group: the first matmul. zr bank: [z | r];
                rx bank: [rh | xh]."""
                zr_ps = ps_zr.tile([128, 2 * HQ], F32, tag="zr", name=f"zr{t}")
                rx_ps = ps_rx.tile([128, 2 * HQ], F32, tag="rx", name=f"rx{t}")
                oh_t = ohw[:, t % win, :]
                for g in range(Q):  # [z|r] one-hot: opens the zr bank
                    nc.tensor.matmul(
                        zr_ps[32 * g : 32 * (g + 1), :],
                        oh_t,
                        gzr[:, g * 2 * HQ : (g + 1) * 2 * HQ],
                        start=True, stop=False, tile_position=(0, 32 * g),
                        skip_group_check=True,
                    )
                for g in range(Q):  # [b1h|xh] one-hot: opens the rx bank
                    nc.tensor.matmul(
                        rx_ps[32 * g : 32 * (g + 1), :],
                        oh_t,
                        grx[:, g * 2 * HQ : (g + 1) * 2 * HQ],
                        start=True, stop=False, tile_position=(0, 32 * g),
                        skip_group_check=True,
                    )
                return zr_ps, rx_ps, oh_t

            def dense_window(nb):
                # dense head for steps [nb*DW, (nb+1)*DW): dw chunks stay
                # stationary, the hT ring slots stream as one N=DW*BQ pass
                dps = ps_dp.tile([VOCAB, DW * BQ], F32, tag="dp", name=f"dp{nb}")
                for c in range(KC):
                    nc.tensor.matmul(
                        dps,
                        dw[:, c, :],
                        hT_roll[:, :, 32 * c : 32 * c + 32],
                        start=(c == 0),
                        stop=False,
                        skip_group_check=True,
                    )
                nc.tensor.matmul(
                    dps, db, ones, start=False, stop=True, skip_group_check=True
                )
                lg = temps.tile([VOCAB, DW * BQ], F32, tag="lg")
                nc.vector.tensor_copy(lg, dps)
                nc.gpsimd.dma_start(out=out_d[nb], in_=lg)

            cur = preissue(0)

            for t in range(n_steps):
                zr_ps, rx_ps, oh_t = cur
                h_prev = h_st[t % 2]
                h_new = h_st[(t + 1) % 2]
                slot_prev = (t - 1) % DW
                slot = t % DW

                if t + pd < n_steps:
                    nc.sync.dma_start(
                        out=ohw[:, (t + pd) % win, :], in_=oh_d[:, t + pd, :]
                    )

                def rec_gate(dst_ps, dcol, wcol):
                    # accumulate h @ Wrec for a full gate; low chunks first
                    # (the half-split transpose writes them earlier)
                    for c in range(KC):
                        last = c == KC - 1
                        for g in range(Q):
                            nc.tensor.matmul(
                                dst_ps[32 * g : 32 * (g + 1), dcol : dcol + HQ],
                                chunk(slot_prev, c),
                                w_all[:, c, wcol + g * HQ : wcol + (g + 1) * HQ],
                                start=False,
                                stop=last,
                                tile_position=(0, 32 * g),
                                skip_group_check=True,
                            )

                # --- r gate ---
                r_t = temps.tile([128, HQ], BF16, tag="r")
                rec_gate(zr_ps, HQ, HID)
                nc.scalar.activation(r_t, zr_ps[:, HQ : 2 * HQ], AF.Sigmoid)

                # --- dense for the previous window: ready immediately, fills
                # the PE while this step's activations run ---
                if t % DW == 0 and t > 0:
                    dense_window(t // DW - 1)

                # --- rh gate; hh = tanh(xh + r*rh); the chain overlaps the z
                # matmuls (full-width: ACT fixed costs make halves slower) ---
                arg = temps.tile([128, HQ], BF16, tag="arg")
                hh = temps.tile([128, HQ], BF16, tag="hh")
                d_t = temps.tile([128, HQ], BF16, tag="d")
                rec_gate(rx_ps, 0, 2 * HID)
                nc.vector.tensor_mul(arg, r_t, rx_ps[:, 0:HQ])
                nc.vector.tensor_add(arg, arg, rx_ps[:, HQ : 2 * HQ])
                nc.scalar.activation(hh, arg, AF.Tanh)
                # d = h_prev - hh: hides under the z sigmoid
                nc.vector.tensor_sub(d_t, h_prev, hh)

                # --- z gate; per half: sigmoid(+mask bias), blend
                # h_new = hh + z*d, block-transpose into the hT ring ---
                z_t = temps.tile([128, HQ], BF16, tag="z")
                e_t = temps.tile([128, HQ], BF16, tag="e")
                rec_gate(zr_ps, 0, 0)
                # first half in column quarters so hT chunks 0-1 land ASAP
                # (rec chunks are consumed in order next step)
                nc.scalar.activation(
                    z_t[:, 0:HH], zr_ps[:, 0:HH], AF.Sigmoid, bias=zb[:, t : t + 1]
                )
                for qtr in range(2):
                    qo = qtr * 64
                    nc.vector.tensor_mul(
                        e_t[:, qo : qo + 64], z_t[:, qo : qo + 64], d_t[:, qo : qo + 64]
                    )
                    nc.vector.tensor_add(
                        h_new[:, qo : qo + 64], hh[:, qo : qo + 64], e_t[:, qo : qo + 64]
                    )
                    nc.vector.transpose(
                        hT_roll[:, slot, qo : qo + 64], h_new[:, qo : qo + 64]
                    )
                nc.scalar.activation(
                    z_t[:, HH : 2 * HH],
                    zr_ps[:, HH : 2 * HH],
                    AF.Sigmoid,
                    bias=zb[:, t : t + 1],
                )
                nc.vector.tensor_mul(
                    e_t[:, HH : 2 * HH], z_t[:, HH : 2 * HH], d_t[:, HH : 2 * HH]
                )
                nc.vector.tensor_add(
                    h_new[:, HH : 2 * HH], hh[:, HH : 2 * HH], e_t[:, HH : 2 * HH]
                )
                nc.vector.transpose(
                    hT_roll[:, slot, HH : 2 * HH], h_new[:, HH : 2 * HH]
                )

                # --- pre-issue step t+1's h-independent matmuls: PE fill for
                # this step's activation tail ---
                if t + 1 < n_steps:
                    cur = preissue(t + 1)

                # --- warmer: semantically-dead matmuls that keep the PE
                # streaming (and clocked up) through the rest of the tail ---
                for w in range(WARM):
                    for g in range(Q):
                        nc.tensor.matmul(
                            warm_ps[32 * g : 32 * (g + 1), 0:HQ],
                            oh_t,
                            gzr[:, g * 2 * HQ : g * 2 * HQ + HQ],
                            start=True, stop=True, tile_position=(0, 32 * g),
                            skip_group_check=True,
                        )

            dense_window(n_steps // DW - 1)

    _split_multiwaits(nc)
    return nc


# ---------------------------------------------------------------------------
# Host-side prep + run
# ---------------------------------------------------------------------------
_CACHE = {}


def _prep_inputs(x, padding_mask, emb_table, gru_kernel, gru_rec_kernel, gru_bias,
                 dense_w, dense_b, n_steps):
    x = np.asarray(x)
    padding_mask = np.asarray(padding_mask)
    emb_table = np.asarray(emb_table, dtype=np.float32)
    gru_kernel = np.asarray(gru_kernel, dtype=np.float32)
    gru_rec_kernel = np.asarray(gru_rec_kernel, dtype=np.float32)
    gru_bias = np.asarray(gru_bias, dtype=np.float32)
    dense_w = np.asarray(dense_w, dtype=np.float32)
    dense_b = np.asarray(dense_b, dtype=np.float32)

    g = emb_table @ gru_kernel  # [VOCAB, 3H]
    g = g + gru_bias[0][None, :]
    g[:, : 2 * HID] += gru_bias[1][None, : 2 * HID]
    b1h = gru_bias[1][None, 2 * HID :].copy()

    def permute_cols(a):
        # per 1024-col gate block: col' = g*256 + u*32 + cc <- 128u + 32g + cc
        # (PE group g owns h-columns {128u+32g+cc}; see kernel docstring)
        blocks = a.reshape(a.shape[0], -1, 8, 4, 32)          # [r, gate, u, g, cc]
        return np.ascontiguousarray(
            np.transpose(blocks, (0, 1, 3, 2, 4)).reshape(a.shape[0], -1)
        )

    gp = permute_cols(g)                       # [VOCAB, 3H], g-grouped cols
    b1p = permute_cols(b1h)                    # [1, HID]
    # per PE group g: gcat_zr = [z_g | r_g]; gcat_rx = [b1h_g | xh_g] with
    # b1h broadcast to every vocab row (one-hot rows sum to 1)
    gz = gp[:, :HID].reshape(VOCAB, Q, HID // Q)
    gr = gp[:, HID : 2 * HID].reshape(VOCAB, Q, HID // Q)
    gx = gp[:, 2 * HID :].reshape(VOCAB, Q, HID // Q)
    bb = np.broadcast_to(b1p, (VOCAB, HID)).reshape(VOCAB, Q, HID // Q)
    gcat_zr = np.concatenate([gz, gr], axis=2).reshape(VOCAB, 2 * HID)
    gcat_rx = np.concatenate([bb, gx], axis=2).reshape(VOCAB, 2 * HID)

    shared = {
        "wrec": permute_cols(gru_rec_kernel).astype(ml_dtypes.bfloat16),
        "gcat_zr": np.ascontiguousarray(gcat_zr).astype(ml_dtypes.bfloat16),
        "gcat_rx": np.ascontiguousarray(gcat_rx).astype(ml_dtypes.bfloat16),
        "dw": np.ascontiguousarray(dense_w).astype(ml_dtypes.bfloat16),
        "db": np.ascontiguousarray(dense_b[None, :]).astype(ml_dtypes.bfloat16),
    }

    in_maps = []
    for c in range(N_CORES):
        q = c % Q
        xs = x[q * BQ : (q + 1) * BQ]
        ms = padding_mask[q * BQ : (q + 1) * BQ]
        onehot = np.zeros((VOCAB, n_steps, BQ), dtype=np.float32)
        tt = np.arange(n_steps)
        for b in range(BQ):
            onehot[xs[b, :n_steps], tt, b] = 1.0
        zbias = np.where(ms[:, :n_steps], 0.0, 30.0).astype(np.float32)  # [BQ, T]
        zbias = np.tile(zbias, (128 // BQ, 1))  # F-layout partitions
        in_maps.append(
            dict(
                shared,
                onehot=onehot.astype(ml_dtypes.bfloat16),
                zbias=np.ascontiguousarray(zbias),
            )
        )
    return in_maps


def kernel(x, padding_mask, emb_table, gru_kernel, gru_rec_kernel, gru_bias,
           dense_w, dense_b, _n_steps: int = T):
    from concourse.bass_utils import run_bass_kernel_spmd

    trace = os.environ.get("BASS_GRU_TRACE", "") == "1"
    if trace:
        _register_axon_profile_hook()

    n_steps = _n_steps
    if n_steps not in _CACHE:
        _CACHE[n_steps] = build_kernel(n_steps)
    nc = _CACHE[n_steps]

    in_maps = _prep_inputs(x, padding_mask, emb_table, gru_kernel, gru_rec_kernel,
                           gru_bias, dense_w, dense_b, n_steps)
    res = run_bass_kernel_spmd(nc, in_maps, list(range(N_CORES)), trace=trace)
    if trace:
        kernel.last_exec_time_ns = res.exec_time_ns
        print(f"HW exec time: {res.exec_time_ns} ns")

    n_dw = (n_steps + DW - 1) // DW
    out = np.empty((B, n_steps, VOCAB), dtype=np.float32)
    for q in range(Q):
        lg = res.results[q]["logits"]  # [n_dw, VOCAB, DW*BQ]
        arr = lg.reshape(n_dw, VOCAB, DW, BQ)
        full = np.transpose(arr, (3, 0, 2, 1)).reshape(BQ, n_dw * DW, VOCAB)
        out[q * BQ : (q + 1) * BQ] = full[:, :n_steps]
    return np.ascontiguousarray(out)


kernel.last_exec_time_ns = None


# revision 51
# speedup vs baseline: 1.0745x; 1.0745x over previous
"""Trainium2 Bass kernel for nn_CasparLayer (embedding -> GRU(reset_after) -> dense).

Problem shapes: B=128, T=256, VOCAB=41, EMB=512, HID=1024.

Strategy (per NeuronCore, SPMD x8):
  - The embedding lookup and the input projection are FUSED into the
    recurrent matmul: x_proj_t = onehot(x_t) @ (emb_table @ gru_kernel + b)
    rides as one extra K=41 chunk of the per-step contraction (one-hot rows
    sum to 1, so per-row-broadcast biases come along for free).
  - Keras masking (h = where(m, h_new, h)) is folded into the z-gate:
    z' = sigmoid(zr + (1-m)*30) == 1 for padded steps, so h carries over.
  - The PE 128x128 array is split into 4 column groups (tile_position=
    (0,32g)), each holding the same [K,32] stationary h-chunk and streaming
    its own quarter of the weight columns - 4 concurrent weight streams at
    M=32.
  - Keeping the PE *continuously* busy is critical: idle gaps drop it out of
    its fast clock state (measured 103ns vs 200ns per 4-group batch of
    N=256). So each step's h-independent matmuls (one-hot x-projections,
    biases) are pre-issued into the previous step's activation tail, gate
    outputs are computed in column halves so sigmoids/tanh/blend pipeline
    with the remaining matmuls, and the transposes/dense fill the rest.

Activations use the 'F-layout': a [32, 1024] quantity is stored as
[128, 256] with partition = 32*(h//256) + b, free = h % 256.

The harness contract: kernel(**inputs) takes full unsharded numpy inputs and
returns the full [128, 256, 41] float32 logits.
"""

import contextlib
import ctypes
import os
import sys
import types

sys.path.insert(0, "/opt/trn_rl_repo")

import numpy as np
import ml_dtypes

import bass_rust
import concourse.bass as bass
import concourse.tile as tile
from concourse import mybir
from concourse.alu_op_type import AluOpType

B = 128
T = 256
VOCAB = 41
EMB = 512
HID = 1024
H3 = 3 * HID
N_CORES = 8
BQ = 32  # batch quarter per core (4-way data parallel, x2 replicas)
Q = 4   # PE column groups
HH = 128  # half of a group's 256-column section (pipelining granule)
OH_WIN = 256  # one-hot SBUF window = all steps: slots written once, no WAR
DW = 8   # dense-head batching window (steps)
WARM = int(os.environ.get("BASS_GRU_WARM", "8"))  # tail warmer batches

F32 = mybir.dt.float32
BF16 = mybir.dt.bfloat16
AF = mybir.ActivationFunctionType


# ---------------------------------------------------------------------------
# Workaround: this walrus build accepts at most ONE sync wait per instruction;
# Tile attaches several. Hoist extras onto single-wait NOPs inserted before.
# ---------------------------------------------------------------------------
def _split_multiwaits(nc, max_waits: int = 1) -> int:
    n_split = 0
    for fn in nc.m.functions:
        for blk in fn.blocks:
            insts = blk.instructions
            i = 0
            while i < len(insts):
                ins = insts[i]
                si = ins.sync_info
                if si is not None and len(si.on_wait) > max_waits:
                    waits = list(si.on_wait)
                    keep = waits[-max_waits:]
                    hoist = waits[:-max_waits]
                    ins.sync_info = bass_rust.SyncInfo(
                        on_wait=keep, on_update=list(si.on_update)
                    )
                    for w in hoist:
                        nop = mybir.InstNoOp(
                            name=nc.get_next_instruction_name(),
                            sync_info=bass_rust.SyncInfo(on_wait=[w], on_update=[]),
                            bass_nofuse=True,
                            engine=ins.engine,
                            text_hint="wait_split",
                        )
                        nc.register_instruction(nop)
                        blk.instructions.insert(i, nop)
                        i += 1
                        n_split += 1
                i += 1
    return n_split


# ---------------------------------------------------------------------------
# Optional NTFF profiling under axon (the container's antenv stub lacks the
# hook registration module). Enabled via BASS_GRU_TRACE=1.
# ---------------------------------------------------------------------------
def _register_axon_profile_hook():
    so_path = "/opt/axon/libaxon_pjrt.so"
    if "antenv.axon_hooks" in sys.modules:
        return
    mod = types.ModuleType("antenv.axon_hooks")
    state = {"hook": None}
    mod.set_axon_ntff_profile_hook = lambda h: state.__setitem__("hook", h)
    mod.get_axon_ntff_profile_hook = lambda: state["hook"]
    sys.modules["antenv.axon_hooks"] = mod

    try:
        lib = ctypes.CDLL(so_path)
    except OSError:
        return
    if not hasattr(lib, "axon_start_nrt_profile"):
        return
    lib.axon_start_nrt_profile.argtypes = [
        ctypes.POINTER(ctypes.c_int64),
        ctypes.c_size_t,
    ]
    lib.axon_start_nrt_profile.restype = ctypes.c_int64
    lib.axon_stop_nrt_profile.argtypes = [ctypes.c_char_p]
    lib.axon_stop_nrt_profile.restype = ctypes.c_int64

    @contextlib.contextmanager
    def _hook_cm(output_dir, device_ids):
        import jax

        jax.devices()
        if device_ids:
            ids = (ctypes.c_int64 * len(device_ids))(*device_ids)
            rc = lib.axon_start_nrt_profile(ids, len(device_ids))
        else:
            rc = lib.axon_start_nrt_profile(None, 0)
        if rc != 0:
            raise RuntimeError(f"axon_start_nrt_profile rc={rc}")
        try:
            yield
        finally:
            n = lib.axon_stop_nrt_profile(str(output_dir).encode())
            print(f"ntff profile: {n} file(s) -> {output_dir}", file=sys.stderr)

    state["hook"] = _hook_cm

    import concourse.bass_utils as bu

    bu.upload_artifacts = lambda tmpdir: ""


# ---------------------------------------------------------------------------
# Kernel builder
# ---------------------------------------------------------------------------
def build_kernel(n_steps: int = T):
    nc = bass.Bass()

    wrec_d = nc.declare_dram_parameter("wrec", [HID, H3], BF16, isOutput=False)
    gzr_d = nc.declare_dram_parameter("gcat_zr", [VOCAB, 2 * HID], BF16, isOutput=False)
    grx_d = nc.declare_dram_parameter("gcat_rx", [VOCAB, 2 * HID], BF16, isOutput=False)
    oh_d = nc.declare_dram_parameter("onehot", [VOCAB, n_steps, BQ], BF16, isOutput=False)
    zb_d = nc.declare_dram_parameter("zbias", [128, n_steps], F32, isOutput=False)
    dw_d = nc.declare_dram_parameter("dw", [HID, VOCAB], BF16, isOutput=False)
    db_d = nc.declare_dram_parameter("db", [1, VOCAB], BF16, isOutput=False)
    n_dw = (n_steps + DW - 1) // DW
    out_d = nc.declare_dram_parameter(
        "logits", [n_dw, VOCAB, DW * BQ], F32, isOutput=True
    )

    KC = HID // 128  # 8 hidden-contraction chunks
    KD = KC // 2
    HQ = HID // Q    # 256 columns per group section

    with tile.TileContext(nc) as tc:
        with contextlib.ExitStack() as ctx:
            singles = ctx.enter_context(tc.tile_pool(name="singles", bufs=1))
            state = ctx.enter_context(tc.tile_pool(name="state", bufs=1))
            temps = ctx.enter_context(tc.tile_pool(name="temps", bufs=2))
            ps_zr = ctx.enter_context(tc.tile_pool(name="ps_zr", bufs=2, space="PSUM"))
            ps_rx = ctx.enter_context(tc.tile_pool(name="ps_rx", bufs=2, space="PSUM"))
            ps_dp = ctx.enter_context(tc.tile_pool(name="ps_dp", bufs=1, space="PSUM"))
            ps_wm = ctx.enter_context(tc.tile_pool(name="ps_wm", bufs=1, space="PSUM"))

            # --- weights / constants resident in SBUF ---
            # Weight columns are host-permuted per gate so PE group g owns the
            # strided h-columns {128u + 32g + cc}: F-layout partition=32g+b,
            # free=32u+cc. A DVE 32x32 block transpose of h then yields hT
            # chunks directly in SBUF (chunk u at free cols 32u:32u+32).
            w_all = singles.tile([128, KC, H3], BF16)
            nc.sync.dma_start(out=w_all, in_=wrec_d.rearrange("(c p) n -> p c n", p=128))
            # one-hot weights, merged per PE group g for single N=512 passes:
            # gcat_zr: [z_g 256 | r_g 256]; gcat_rx: [b1h_g 256 | xh_g 256]
            # (b1h rows replicated per vocab entry: one-hot rows sum to 1)
            gzr = singles.tile([VOCAB, 2 * HID], BF16)
            nc.sync.dma_start(out=gzr, in_=gzr_d[:])
            grx = singles.tile([VOCAB, 2 * HID], BF16)
            nc.sync.dma_start(out=grx, in_=grx_d[:])
            dw = singles.tile([128, KC, VOCAB], BF16)
            nc.sync.dma_start(out=dw, in_=dw_d.rearrange("(c p) n -> p c n", p=128))
            db = singles.tile([1, VOCAB], BF16)
            nc.sync.dma_start(out=db, in_=db_d[:])
            zb = singles.tile([128, n_steps], F32)
            nc.sync.dma_start(out=zb, in_=zb_d[:])
            ones = singles.tile([1, DW * BQ], BF16)
            nc.vector.memset(ones, 1.0)

            # one-hot window streamed via the sync DMA queue (logits go out on
            # the gpsimd queue so they never block these)
            win = min(OH_WIN, n_steps)
            pd = max(1, win // 2)
            ohw = singles.tile([VOCAB, win, BQ], BF16)
            for t in range(min(pd, n_steps)):
                nc.sync.dma_start(out=ohw[:, t % win, :], in_=oh_d[:, t, :])

            # --- GRU state: h in F-layout (ping-pong); hT in a DW-deep ring
            # of [128, 256] slots (chunk u at free cols 32u:32u+32); the ring
            # depth lets the dense head batch DW steps at once. ---
            h_st = [
                state.tile([128, HQ], BF16, tag=f"h{i}", name=f"h{i}") for i in range(2)
            ]
            hT_roll = state.tile([128, DW, HQ], BF16, tag="hTr", name="hTr")
            nc.vector.memset(h_st[0], 0.0)
            nc.vector.memset(hT_roll, 0.0)
            warm_ps = ps_wm.tile([128, 2 * HQ], F32, tag="warm", name="warm")

            def chunk(slot, c):
                return hT_roll[:, slot, 32 * c : 32 * c + 32]

            def preissue(t):
                """h-independent matmuls for step t: one-hot x-projections
                (embedding+input-proj fused, includes b0 and b1[z,r]) and the
                rh-gate b1h bias. These fill the PE during step t-1's
                activation tail and open every accumulation region.

                PSUM start=True zeroes the ENTIRE 2KB bank row for the
                written partitions, so exactly ONE start per bank per
                partition-group: the first matmul. zr bank: [z | r];
                rx bank: [rh | xh]."""
                zr_ps = ps_zr.tile([128, 2 * HQ], F32, tag="zr", name=f"zr{t}")
                rx_ps = ps_rx.tile([128, 2 * HQ], F32, tag="rx", name=f"rx{t}")
                oh_t = ohw[:, t % win, :]
                for g in range(Q):  # [z|r] one-hot: opens the zr bank
                    nc.tensor.matmul(
                        zr_ps[32 * g : 32 * (g + 1), :],
                        oh_t,
                        gzr[:, g * 2 * HQ : (g + 1) * 2 * HQ],
                        start=True, stop=False, tile_position=(0, 32 * g),
                        skip_group_check=True,
                    )
                for g in range(Q):  # [b1h|xh] one-hot: opens the rx bank
                    nc.tensor.matmul(
                        rx_ps[32 * g : 32 * (g + 1), :],
                        oh_t,
                        grx[:, g * 2 * HQ : (g + 1) * 2 * HQ],
                        start=True, stop=False, tile_position=(0, 32 * g),
                        skip_group_check=True,
                    )
                return zr_ps, rx_ps, oh_t

            def dense_window(nb):
                # dense head for steps [nb*DW, (nb+1)*DW): dw chunks stay
                # stationary, the hT ring slots stream as one N=DW*BQ pass
                dps = ps_dp.tile([VOCAB, DW * BQ], F32, tag="dp", name=f"dp{nb}")
                for c in range(KC):
                    nc.tensor.matmul(
                        dps,
                        dw[:, c, :],
                        hT_roll[:, :, 32 * c : 32 * c + 32],
                        start=(c == 0),
                        stop=False,
                        skip_group_check=True,
                    )
                nc.tensor.matmul(
                    dps, db, ones, start=False, stop=True, skip_group_check=True
                )
                lg = temps.tile([VOCAB, DW * BQ], F32, tag="lg")
                nc.vector.tensor_copy(lg, dps)
                nc.gpsimd.dma_start(out=out_d[nb], in_=lg)

            cur = preissue(0)

            for t in range(n_steps):
                zr_ps, rx_ps, oh_t = cur
                h_prev = h_st[t % 2]
                h_new = h_st[(t + 1) % 2]
                slot_prev = (t - 1) % DW
                slot = t % DW

                if t + pd < n_steps:
                    nc.sync.dma_start(
                        out=ohw[:, (t + pd) % win, :], in_=oh_d[:, t + pd, :]
                    )

                def rec_gate(dst_ps, dcol, wcol):
                    # accumulate h @ Wrec for a full gate; low chunks first
                    # (the half-split transpose writes them earlier)
                    for c in range(KC):
                        last = c == KC - 1
                        for g in range(Q):
                            nc.tensor.matmul(
                                dst_ps[32 * g : 32 * (g + 1), dcol : dcol + HQ],
                                chunk(slot_prev, c),
                                w_all[:, c, wcol + g * HQ : wcol + (g + 1) * HQ],
                                start=False,
                                stop=last,
                                tile_position=(0, 32 * g),
                                skip_group_check=True,
                            )

                # --- r gate ---
                r_t = temps.tile([128, HQ], BF16, tag="r")
                rec_gate(zr_ps, HQ, HID)
                nc.scalar.activation(r_t, zr_ps[:, HQ : 2 * HQ], AF.Sigmoid)

                # --- dense for the previous window: ready immediately, fills
                # the PE while this step's activations run ---
                if t % DW == 0 and t > 0:
                    dense_window(t // DW - 1)

                # --- rh gate; hh = tanh(xh + r*rh); the chain overlaps the z
                # matmuls (full-width: ACT fixed costs make halves slower) ---
                arg = temps.tile([128, HQ], BF16, tag="arg")
                hh = temps.tile([128, HQ], BF16, tag="hh")
                d_t = temps.tile([128, HQ], BF16, tag="d")
                rec_gate(rx_ps, 0, 2 * HID)
                nc.vector.tensor_mul(arg, r_t, rx_ps[:, 0:HQ])
                nc.vector.tensor_add(arg, arg, rx_ps[:, HQ : 2 * HQ])
                nc.scalar.activation(hh, arg, AF.Tanh)
                # d = h_prev - hh: hides under the z sigmoid
                nc.vector.tensor_sub(d_t, h_prev, hh)

                # --- z gate; per half: sigmoid(+mask bias), blend
                # h_new = hh + z*d, block-transpose into the hT ring ---
                z_t = temps.tile([128, HQ], BF16, tag="z")
                e_t = temps.tile([128, HQ], BF16, tag="e")
                rec_gate(zr_ps, 0, 0)
                for s in range(2):
                    so = s * HH
                    nc.scalar.activation(
                        z_t[:, so : so + HH],
                        zr_ps[:, so : so + HH],
                        AF.Sigmoid,
                        bias=zb[:, t : t + 1],
                    )
                    nc.vector.tensor_mul(
                        e_t[:, so : so + HH], z_t[:, so : so + HH], d_t[:, so : so + HH]
                    )
                    nc.vector.tensor_add(
                        h_new[:, so : so + HH], hh[:, so : so + HH], e_t[:, so : so + HH]
                    )
                    nc.vector.transpose(
                        hT_roll[:, slot, so : so + HH], h_new[:, so : so + HH]
                    )

                # --- pre-issue step t+1's h-independent matmuls: PE fill for
                # this step's activation tail ---
                if t + 1 < n_steps:
                    cur = preissue(t + 1)

                # --- warmer: semantically-dead matmuls that keep the PE
                # streaming (and clocked up) through the rest of the tail ---
                for w in range(WARM):
                    for g in range(Q):
                        nc.tensor.matmul(
                            warm_ps[32 * g : 32 * (g + 1), 0:HQ],
                            oh_t,
                            gzr[:, g * 2 * HQ : g * 2 * HQ + HQ],
                            start=True, stop=True, tile_position=(0, 32 * g),
                            skip_group_check=True,
                        )

            dense_window(n_steps // DW - 1)

    _split_multiwaits(nc)
    return nc


# ---------------------------------------------------------------------------
# Host-side prep + run
# ---------------------------------------------------------------------------
_CACHE = {}


def _prep_inputs(x, padding_mask, emb_table, gru_kernel, gru_rec_kernel, gru_bias,
                 dense_w, dense_b, n_steps):
    x = np.asarray(x)
    padding_mask = np.asarray(padding_mask)
    emb_table = np.asarray(emb_table, dtype=np.float32)
    gru_kernel = np.asarray(gru_kernel, dtype=np.float32)
    gru_rec_kernel = np.asarray(gru_rec_kernel, dtype=np.float32)
    gru_bias = np.asarray(gru_bias, dtype=np.float32)
    dense_w = np.asarray(dense_w, dtype=np.float32)
    dense_b = np.asarray(dense_b, dtype=np.float32)

    g = emb_table @ gru_kernel  # [VOCAB, 3H]
    g = g + gru_bias[0][None, :]
    g[:, : 2 * HID] += gru_bias[1][None, : 2 * HID]
    b1h = gru_bias[1][None, 2 * HID :].copy()

    def permute_cols(a):
        # per 1024-col gate block: col' = g*256 + u*32 + cc <- 128u + 32g + cc
        # (PE group g owns h-columns {128u+32g+cc}; see kernel docstring)
        blocks = a.reshape(a.shape[0], -1, 8, 4, 32)          # [r, gate, u, g, cc]
        return np.ascontiguousarray(
            np.transpose(blocks, (0, 1, 3, 2, 4)).reshape(a.shape[0], -1)
        )

    gp = permute_cols(g)                       # [VOCAB, 3H], g-grouped cols
    b1p = permute_cols(b1h)                    # [1, HID]
    # per PE group g: gcat_zr = [z_g | r_g]; gcat_rx = [b1h_g | xh_g] with
    # b1h broadcast to every vocab row (one-hot rows sum to 1)
    gz = gp[:, :HID].reshape(VOCAB, Q, HID // Q)
    gr = gp[:, HID : 2 * HID].reshape(VOCAB, Q, HID // Q)
    gx = gp[:, 2 * HID :].reshape(VOCAB, Q, HID // Q)
    bb = np.broadcast_to(b1p, (VOCAB, HID)).reshape(VOCAB, Q, HID // Q)
    gcat_zr = np.concatenate([gz, gr], axis=2).reshape(VOCAB, 2 * HID)
    gcat_rx = np.concatenate([bb, gx], axis=2).reshape(VOCAB, 2 * HID)

    shared = {
        "wrec": permute_cols(gru_rec_kernel).astype(ml_dtypes.bfloat16),
        "gcat_zr": np.ascontiguousarray(gcat_zr).astype(ml_dtypes.bfloat16),
        "gcat_rx": np.ascontiguousarray(gcat_rx).astype(ml_dtypes.bfloat16),
        "dw": np.ascontiguousarray(dense_w).astype(ml_dtypes.bfloat16),
        "db": np.ascontiguousarray(dense_b[None, :]).astype(ml_dtypes.bfloat16),
    }

    in_maps = []
    for c in range(N_CORES):
        q = c % Q
        xs = x[q * BQ : (q + 1) * BQ]
        ms = padding_mask[q * BQ : (q + 1) * BQ]
        onehot = np.zeros((VOCAB, n_steps, BQ), dtype=np.float32)
        tt = np.arange(n_steps)
        for b in range(BQ):
            onehot[xs[b, :n_steps], tt, b] = 1.0
        zbias = np.where(ms[:, :n_steps], 0.0, 30.0).astype(np.float32)  # [BQ, T]
        zbias = np.tile(zbias, (128 // BQ, 1))  # F-layout partitions
        in_maps.append(
            dict(
                shared,
                onehot=onehot.astype(ml_dtypes.bfloat16),
                zbias=np.ascontiguousarray(zbias),
            )
        )
    return in_maps


def kernel(x, padding_mask, emb_table, gru_kernel, gru_rec_kernel, gru_bias,
           dense_w, dense_b, _n_steps: int = T):
    from concourse.bass_utils import run_bass_kernel_spmd

    trace = os.environ.get("BASS_GRU_TRACE", "") == "1"
    if trace:
        _register_axon_profile_hook()

    n_steps = _n_steps
    if n_steps not in _CACHE:
        _CACHE[n_steps] = build_kernel(n_steps)
    nc = _CACHE[n_steps]

    in_maps = _prep_inputs(x, padding_mask, emb_table, gru_kernel, gru_rec_kernel,
                           gru_bias, dense_w, dense_b, n_steps)
    res = run_bass_kernel_spmd(nc, in_maps, list(range(N_CORES)), trace=trace)
    if trace:
        kernel.last_exec_time_ns = res.exec_time_ns
        print(f"HW exec time: {res.exec_time_ns} ns")

    n_dw = (n_steps + DW - 1) // DW
    out = np.empty((B, n_steps, VOCAB), dtype=np.float32)
    for q in range(Q):
        lg = res.results[q]["logits"]  # [n_dw, VOCAB, DW*BQ]
        arr = lg.reshape(n_dw, VOCAB, DW, BQ)
        full = np.transpose(arr, (3, 0, 2, 1)).reshape(BQ, n_dw * DW, VOCAB)
        out[q * BQ : (q + 1) * BQ] = full[:, :n_steps]
    return np.ascontiguousarray(out)


kernel.last_exec_time_ns = None


# revision 52
# speedup vs baseline: 1.0745x; 1.0001x over previous
"""Trainium2 Bass kernel for nn_CasparLayer (embedding -> GRU(reset_after) -> dense).

Problem shapes: B=128, T=256, VOCAB=41, EMB=512, HID=1024.

Strategy (per NeuronCore, SPMD x8):
  - The embedding lookup and the input projection are FUSED into the
    recurrent matmul: x_proj_t = onehot(x_t) @ (emb_table @ gru_kernel + b)
    rides as one extra K=41 chunk of the per-step contraction (one-hot rows
    sum to 1, so per-row-broadcast biases come along for free).
  - Keras masking (h = where(m, h_new, h)) is folded into the z-gate:
    z' = sigmoid(zr + (1-m)*30) == 1 for padded steps, so h carries over.
  - The PE 128x128 array is split into 4 column groups (tile_position=
    (0,32g)), each holding the same [K,32] stationary h-chunk and streaming
    its own quarter of the weight columns - 4 concurrent weight streams at
    M=32 (measured: a 4-group batch of N=256 retires every ~103ns at the
    PE's fast clock state).
  - Weight columns are host-permuted so group g owns the strided h-columns
    {128u+32g+cc} ('F-layout': partition = 32g + batch, free = 32u + cc).
    A single DVE 32x32 block transpose of h_new then yields ALL hT chunks
    directly in SBUF (chunk u at free cols 32u:32u+32) - no PE transpose,
    no PSUM->SBUF copies on the recurrence critical path.
  - Keeping the PE *continuously* busy is critical: idle gaps drop it out
    of its fast clock state (~2x slower rows). Each step's h-independent
    matmuls (one-hot x-projections + biases, merged into N=512 passes) are
    pre-issued into the previous step's activation tail, the dense head is
    batched over DW=8 steps against an hT ring buffer and runs one window
    late (pure fill), and a few semantically-dead 'warmer' matmuls bridge
    the rest of the tail.
  - The one-hot window covers all 256 steps (each SBUF slot written once,
    no WAR) and logits leave via the gpsimd DMA queue, so the per-step
    matmuls never wait on DMA semaphores.
  - PSUM start=True zeroes the whole 2KB bank row for the written
    partitions, so exactly one opener per bank per step (the pre-issued
    one-hot matmul), everything else accumulates with start=False.

The harness contract: kernel(**inputs) takes full unsharded numpy inputs and
returns the full [128, 256, 41] float32 logits.
"""

import contextlib
import ctypes
import os
import sys
import types

sys.path.insert(0, "/opt/trn_rl_repo")

import numpy as np
import ml_dtypes

import bass_rust
import concourse.bass as bass
import concourse.tile as tile
from concourse import mybir
from concourse.alu_op_type import AluOpType

B = 128
T = 256
VOCAB = 41
EMB = 512
HID = 1024
H3 = 3 * HID
N_CORES = 8
BQ = 32  # batch quarter per core (4-way data parallel, x2 replicas)
Q = 4   # PE column groups
HH = 128  # half of a group's 256-column section (pipelining granule)
OH_WIN = 256  # one-hot SBUF window = all steps: slots written once, no WAR
DW = 8   # dense-head batching window (steps)
WARM = int(os.environ.get("BASS_GRU_WARM", "8"))  # tail warmer batches

F32 = mybir.dt.float32
BF16 = mybir.dt.bfloat16
AF = mybir.ActivationFunctionType


# ---------------------------------------------------------------------------
# Workaround: this walrus build accepts at most ONE sync wait per instruction;
# Tile attaches several. Hoist extras onto single-wait NOPs inserted before.
# ---------------------------------------------------------------------------
def _split_multiwaits(nc, max_waits: int = 1) -> int:
    n_split = 0
    for fn in nc.m.functions:
        for blk in fn.blocks:
            insts = blk.instructions
            i = 0
            while i < len(insts):
                ins = insts[i]
                si = ins.sync_info
                if si is not None and len(si.on_wait) > max_waits:
                    waits = list(si.on_wait)
                    keep = waits[-max_waits:]
                    hoist = waits[:-max_waits]
                    ins.sync_info = bass_rust.SyncInfo(
                        on_wait=keep, on_update=list(si.on_update)
                    )
                    for w in hoist:
                        nop = mybir.InstNoOp(
                            name=nc.get_next_instruction_name(),
                            sync_info=bass_rust.SyncInfo(on_wait=[w], on_update=[]),
                            bass_nofuse=True,
                            engine=ins.engine,
                            text_hint="wait_split",
                        )
                        nc.register_instruction(nop)
                        blk.instructions.insert(i, nop)
                        i += 1
                        n_split += 1
                i += 1
    return n_split


# ---------------------------------------------------------------------------
# Optional NTFF profiling under axon (the container's antenv stub lacks the
# hook registration module). Enabled via BASS_GRU_TRACE=1.
# ---------------------------------------------------------------------------
def _register_axon_profile_hook():
    so_path = "/opt/axon/libaxon_pjrt.so"
    if "antenv.axon_hooks" in sys.modules:
        return
    mod = types.ModuleType("antenv.axon_hooks")
    state = {"hook": None}
    mod.set_axon_ntff_profile_hook = lambda h: state.__setitem__("hook", h)
    mod.get_axon_ntff_profile_hook = lambda: state["hook"]
    sys.modules["antenv.axon_hooks"] = mod

    try:
        lib = ctypes.CDLL(so_path)
    except OSError:
        return
    if not hasattr(lib, "axon_start_nrt_profile"):
        return
    lib.axon_start_nrt_profile.argtypes = [
        ctypes.POINTER(ctypes.c_int64),
        ctypes.c_size_t,
    ]
    lib.axon_start_nrt_profile.restype = ctypes.c_int64
    lib.axon_stop_nrt_profile.argtypes = [ctypes.c_char_p]
    lib.axon_stop_nrt_profile.restype = ctypes.c_int64

    @contextlib.contextmanager
    def _hook_cm(output_dir, device_ids):
        import jax

        jax.devices()
        if device_ids:
            ids = (ctypes.c_int64 * len(device_ids))(*device_ids)
            rc = lib.axon_start_nrt_profile(ids, len(device_ids))
        else:
            rc = lib.axon_start_nrt_profile(None, 0)
        if rc != 0:
            raise RuntimeError(f"axon_start_nrt_profile rc={rc}")
        try:
            yield
        finally:
            n = lib.axon_stop_nrt_profile(str(output_dir).encode())
            print(f"ntff profile: {n} file(s) -> {output_dir}", file=sys.stderr)

    state["hook"] = _hook_cm

    import concourse.bass_utils as bu

    bu.upload_artifacts = lambda tmpdir: ""


# ---------------------------------------------------------------------------
# Kernel builder
# ---------------------------------------------------------------------------
def build_kernel(n_steps: int = T):
    nc = bass.Bass()

    wrec_d = nc.declare_dram_parameter("wrec", [HID, H3], BF16, isOutput=False)
    gzr_d = nc.declare_dram_parameter("gcat_zr", [VOCAB, 2 * HID], BF16, isOutput=False)
    grx_d = nc.declare_dram_parameter("gcat_rx", [VOCAB, 2 * HID], BF16, isOutput=False)
    oh_d = nc.declare_dram_parameter("onehot", [VOCAB, n_steps, BQ], BF16, isOutput=False)
    zb_d = nc.declare_dram_parameter("zbias", [128, n_steps], F32, isOutput=False)
    dw_d = nc.declare_dram_parameter("dw", [HID, VOCAB], BF16, isOutput=False)
    db_d = nc.declare_dram_parameter("db", [1, VOCAB], BF16, isOutput=False)
    n_dw = (n_steps + DW - 1) // DW
    out_d = nc.declare_dram_parameter(
        "logits", [n_dw, VOCAB, DW * BQ], F32, isOutput=True
    )

    KC = HID // 128  # 8 hidden-contraction chunks
    KD = KC // 2
    HQ = HID // Q    # 256 columns per group section

    with tile.TileContext(nc) as tc:
        with contextlib.ExitStack() as ctx:
            singles = ctx.enter_context(tc.tile_pool(name="singles", bufs=1))
            state = ctx.enter_context(tc.tile_pool(name="state", bufs=1))
            temps = ctx.enter_context(tc.tile_pool(name="temps", bufs=2))
            ps_zr = ctx.enter_context(tc.tile_pool(name="ps_zr", bufs=2, space="PSUM"))
            ps_rx = ctx.enter_context(tc.tile_pool(name="ps_rx", bufs=2, space="PSUM"))
            ps_dp = ctx.enter_context(tc.tile_pool(name="ps_dp", bufs=1, space="PSUM"))
            ps_wm = ctx.enter_context(tc.tile_pool(name="ps_wm", bufs=1, space="PSUM"))

            # --- weights / constants resident in SBUF ---
            # Weight columns are host-permuted per gate so PE group g owns the
            # strided h-columns {128u + 32g + cc}: F-layout partition=32g+b,
            # free=32u+cc. A DVE 32x32 block transpose of h then yields hT
            # chunks directly in SBUF (chunk u at free cols 32u:32u+32).
            w_all = singles.tile([128, KC, H3], BF16)
            nc.sync.dma_start(out=w_all, in_=wrec_d.rearrange("(c p) n -> p c n", p=128))
            # one-hot weights, merged per PE group g for single N=512 passes:
            # gcat_zr: [z_g 256 | r_g 256]; gcat_rx: [b1h_g 256 | xh_g 256]
            # (b1h rows replicated per vocab entry: one-hot rows sum to 1)
            gzr = singles.tile([VOCAB, 2 * HID], BF16)
            nc.sync.dma_start(out=gzr, in_=gzr_d[:])
            grx = singles.tile([VOCAB, 2 * HID], BF16)
            nc.sync.dma_start(out=grx, in_=grx_d[:])
            dw = singles.tile([128, KC, VOCAB], BF16)
            nc.sync.dma_start(out=dw, in_=dw_d.rearrange("(c p) n -> p c n", p=128))
            db = singles.tile([1, VOCAB], BF16)
            nc.sync.dma_start(out=db, in_=db_d[:])
            zb = singles.tile([128, n_steps], F32)
            nc.sync.dma_start(out=zb, in_=zb_d[:])
            ones = singles.tile([1, DW * BQ], BF16)
            nc.vector.memset(ones, 1.0)

            # one-hot window streamed via the sync DMA queue (logits go out on
            # the gpsimd queue so they never block these)
            win = min(OH_WIN, n_steps)
            pd = max(1, win // 2)
            ohw = singles.tile([VOCAB, win, BQ], BF16)
            for t in range(min(pd, n_steps)):
                nc.sync.dma_start(out=ohw[:, t % win, :], in_=oh_d[:, t, :])

            # --- GRU state: h in F-layout (ping-pong); hT in a DW-deep ring
            # of [128, 256] slots (chunk u at free cols 32u:32u+32); the ring
            # depth lets the dense head batch DW steps at once. ---
            h_st = [
                state.tile([128, HQ], BF16, tag=f"h{i}", name=f"h{i}") for i in range(2)
            ]
            hT_roll = state.tile([128, DW, HQ], BF16, tag="hTr", name="hTr")
            nc.vector.memset(h_st[0], 0.0)
            nc.vector.memset(hT_roll, 0.0)
            warm_ps = ps_wm.tile([128, 2 * HQ], F32, tag="warm", name="warm")

            def chunk(slot, c):
                return hT_roll[:, slot, 32 * c : 32 * c + 32]

            def preissue(t):
                """h-independent matmuls for step t: one-hot x-projections
                (embedding+input-proj fused, includes b0 and b1[z,r]) and the
                rh-gate b1h bias. These fill the PE during step t-1's
                activation tail and open every accumulation region.

                PSUM start=True zeroes the ENTIRE 2KB bank row for the
                written partitions, so exactly ONE start per bank per
                partition-group: the first matmul. zr bank: [z | r];
                rx bank: [rh | xh]."""
                zr_ps = ps_zr.tile([128, 2 * HQ], F32, tag="zr", name=f"zr{t}")
                rx_ps = ps_rx.tile([128, 2 * HQ], F32, tag="rx", name=f"rx{t}")
                oh_t = ohw[:, t % win, :]
                for g in range(Q):  # [z|r] one-hot: opens the zr bank
                    nc.tensor.matmul(
                        zr_ps[32 * g : 32 * (g + 1), :],
                        oh_t,
                        gzr[:, g * 2 * HQ : (g + 1) * 2 * HQ],
                        start=True, stop=False, tile_position=(0, 32 * g),
                        skip_group_check=True,
                    )
                for g in range(Q):  # [b1h|xh] one-hot: opens the rx bank
                    nc.tensor.matmul(
                        rx_ps[32 * g : 32 * (g + 1), :],
                        oh_t,
                        grx[:, g * 2 * HQ : (g + 1) * 2 * HQ],
                        start=True, stop=False, tile_position=(0, 32 * g),
                        skip_group_check=True,
                    )
                return zr_ps, rx_ps, oh_t

            def dense_window(nb):
                # dense head for steps [nb*DW, (nb+1)*DW): dw chunks stay
                # stationary, the hT ring slots stream as one N=DW*BQ pass
                dps = ps_dp.tile([VOCAB, DW * BQ], F32, tag="dp", name=f"dp{nb}")
                for c in range(KC):
                    nc.tensor.matmul(
                        dps,
                        dw[:, c, :],
                        hT_roll[:, :, 32 * c : 32 * c + 32],
                        start=(c == 0),
                        stop=False,
                        skip_group_check=True,
                    )
                nc.tensor.matmul(
                    dps, db, ones, start=False, stop=True, skip_group_check=True
                )
                lg = temps.tile([VOCAB, DW * BQ], F32, tag="lg")
                nc.vector.tensor_copy(lg, dps)
                nc.gpsimd.dma_start(out=out_d[nb], in_=lg)

            cur = preissue(0)

            for t in range(n_steps):
                zr_ps, rx_ps, oh_t = cur
                h_prev = h_st[t % 2]
                h_new = h_st[(t + 1) % 2]
                slot_prev = (t - 1) % DW
                slot = t % DW

                if t + pd < n_steps:
                    nc.sync.dma_start(
                        out=ohw[:, (t + pd) % win, :], in_=oh_d[:, t + pd, :]
                    )

                def rec_gate(dst_ps, dcol, wcol):
                    # accumulate h @ Wrec for a full gate; low chunks first
                    # (the half-split transpose writes them earlier)
                    for c in range(KC):
                        last = c == KC - 1
                        for g in range(Q):
                            nc.tensor.matmul(
                                dst_ps[32 * g : 32 * (g + 1), dcol : dcol + HQ],
                                chunk(slot_prev, c),
                                w_all[:, c, wcol + g * HQ : wcol + (g + 1) * HQ],
                                start=False,
                                stop=last,
                                tile_position=(0, 32 * g),
                                skip_group_check=True,
                            )

                # --- r gate ---
                r_t = temps.tile([128, HQ], BF16, tag="r")
                rec_gate(zr_ps, HQ, HID)
                nc.scalar.activation(r_t, zr_ps[:, HQ : 2 * HQ], AF.Sigmoid)

                # --- dense for the previous window: ready immediately, fills
                # the PE while this step's activations run ---
                if t % DW == 0 and t > 0:
                    dense_window(t // DW - 1)

                # --- rh gate; hh = tanh(xh + r*rh); the chain overlaps the z
                # matmuls (full-width: ACT fixed costs make halves slower) ---
                arg = temps.tile([128, HQ], BF16, tag="arg")
                hh = temps.tile([128, HQ], BF16, tag="hh")
                d_t = temps.tile([128, HQ], BF16, tag="d")
                rec_gate(rx_ps, 0, 2 * HID)
                nc.vector.tensor_mul(arg, r_t, rx_ps[:, 0:HQ])
                nc.vector.tensor_add(arg, arg, rx_ps[:, HQ : 2 * HQ])
                nc.scalar.activation(hh, arg, AF.Tanh)
                # d = h_prev - hh: hides under the z sigmoid
                nc.vector.tensor_sub(d_t, h_prev, hh)

                # --- z gate; per half: sigmoid(+mask bias), blend
                # h_new = hh + z*d, block-transpose into the hT ring ---
                z_t = temps.tile([128, HQ], BF16, tag="z")
                e_t = temps.tile([128, HQ], BF16, tag="e")
                rec_gate(zr_ps, 0, 0)
                for s in range(2):
                    so = s * HH
                    nc.scalar.activation(
                        z_t[:, so : so + HH],
                        zr_ps[:, so : so + HH],
                        AF.Sigmoid,
                        bias=zb[:, t : t + 1],
                    )
                    nc.vector.tensor_mul(
                        e_t[:, so : so + HH], z_t[:, so : so + HH], d_t[:, so : so + HH]
                    )
                    nc.vector.tensor_add(
                        h_new[:, so : so + HH], hh[:, so : so + HH], e_t[:, so : so + HH]
                    )
                    nc.vector.transpose(
                        hT_roll[:, slot, so : so + HH], h_new[:, so : so + HH]
                    )

                # --- pre-issue step t+1's h-independent matmuls: PE fill for
                # this step's activation tail ---
                if t + 1 < n_steps:
                    cur = preissue(t + 1)

                # --- warmer: semantically-dead matmuls that keep the PE
                # streaming (and clocked up) through the rest of the tail ---
                for w in range(WARM):
                    for g in range(Q):
                        nc.tensor.matmul(
                            warm_ps[32 * g : 32 * (g + 1), 0:HQ],
                            oh_t,
                            gzr[:, g * 2 * HQ : g * 2 * HQ + HQ],
                            start=True, stop=True, tile_position=(0, 32 * g),
                            skip_group_check=True,
                        )

            dense_window(n_steps // DW - 1)

    _split_multiwaits(nc)
    return nc


# ---------------------------------------------------------------------------
# Host-side prep + run
# ---------------------------------------------------------------------------
_CACHE = {}


def _prep_inputs(x, padding_mask, emb_table, gru_kernel, gru_rec_kernel, gru_bias,
                 dense_w, dense_b, n_steps):
    x = np.asarray(x)
    padding_mask = np.asarray(padding_mask)
    emb_table = np.asarray(emb_table, dtype=np.float32)
    gru_kernel = np.asarray(gru_kernel, dtype=np.float32)
    gru_rec_kernel = np.asarray(gru_rec_kernel, dtype=np.float32)
    gru_bias = np.asarray(gru_bias, dtype=np.float32)
    dense_w = np.asarray(dense_w, dtype=np.float32)
    dense_b = np.asarray(dense_b, dtype=np.float32)

    g = emb_table @ gru_kernel  # [VOCAB, 3H]
    g = g + gru_bias[0][None, :]
    g[:, : 2 * HID] += gru_bias[1][None, : 2 * HID]
    b1h = gru_bias[1][None, 2 * HID :].copy()

    def permute_cols(a):
        # per 1024-col gate block: col' = g*256 + u*32 + cc <- 128u + 32g + cc
        # (PE group g owns h-columns {128u+32g+cc}; see kernel docstring)
        blocks = a.reshape(a.shape[0], -1, 8, 4, 32)          # [r, gate, u, g, cc]
        return np.ascontiguousarray(
            np.transpose(blocks, (0, 1, 3, 2, 4)).reshape(a.shape[0], -1)
        )

    gp = permute_cols(g)                       # [VOCAB, 3H], g-grouped cols
    b1p = permute_cols(b1h)                    # [1, HID]
    # per PE group g: gcat_zr = [z_g | r_g]; gcat_rx = [b1h_g | xh_g] with
    # b1h broadcast to every vocab row (one-hot rows sum to 1)
    gz = gp[:, :HID].reshape(VOCAB, Q, HID // Q)
    gr = gp[:, HID : 2 * HID].reshape(VOCAB, Q, HID // Q)
    gx = gp[:, 2 * HID :].reshape(VOCAB, Q, HID // Q)
    bb = np.broadcast_to(b1p, (VOCAB, HID)).reshape(VOCAB, Q, HID // Q)
    gcat_zr = np.concatenate([gz, gr], axis=2).reshape(VOCAB, 2 * HID)
    gcat_rx = np.concatenate([bb, gx], axis=2).reshape(VOCAB, 2 * HID)

    shared = {
        "wrec": permute_cols(gru_rec_kernel).astype(ml_dtypes.bfloat16),
        "gcat_zr": np.ascontiguousarray(gcat_zr).astype(ml_dtypes.bfloat16),
        "gcat_rx": np.ascontiguousarray(gcat_rx).astype(ml_dtypes.bfloat16),
        "dw": np.ascontiguousarray(dense_w).astype(ml_dtypes.bfloat16),
        "db": np.ascontiguousarray(dense_b[None, :]).astype(ml_dtypes.bfloat16),
    }

    in_maps = []
    for c in range(N_CORES):
        q = c % Q
        xs = x[q * BQ : (q + 1) * BQ]
        ms = padding_mask[q * BQ : (q + 1) * BQ]
        onehot = np.zeros((VOCAB, n_steps, BQ), dtype=np.float32)
        tt = np.arange(n_steps)
        for b in range(BQ):
            onehot[xs[b, :n_steps], tt, b] = 1.0
        zbias = np.where(ms[:, :n_steps], 0.0, 30.0).astype(np.float32)  # [BQ, T]
        zbias = np.tile(zbias, (128 // BQ, 1))  # F-layout partitions
        in_maps.append(
            dict(
                shared,
                onehot=onehot.astype(ml_dtypes.bfloat16),
                zbias=np.ascontiguousarray(zbias),
            )
        )
    return in_maps


def kernel(x, padding_mask, emb_table, gru_kernel, gru_rec_kernel, gru_bias,
           dense_w, dense_b, _n_steps: int = T):
    from concourse.bass_utils import run_bass_kernel_spmd

    trace = os.environ.get("BASS_GRU_TRACE", "") == "1"
    if trace:
        _register_axon_profile_hook()

    n_steps = _n_steps
    if n_steps not in _CACHE:
        _CACHE[n_steps] = build_kernel(n_steps)
    nc = _CACHE[n_steps]

    in_maps = _prep_inputs(x, padding_mask, emb_table, gru_kernel, gru_rec_kernel,
                           gru_bias, dense_w, dense_b, n_steps)
    res = run_bass_kernel_spmd(nc, in_maps, list(range(N_CORES)), trace=trace)
    if trace:
        kernel.last_exec_time_ns = res.exec_time_ns
        print(f"HW exec time: {res.exec_time_ns} ns")

    n_dw = (n_steps + DW - 1) // DW
    out = np.empty((B, n_steps, VOCAB), dtype=np.float32)
    for q in range(Q):
        lg = res.results[q]["logits"]  # [n_dw, VOCAB, DW*BQ]
        arr = lg.reshape(n_dw, VOCAB, DW, BQ)
        full = np.transpose(arr, (3, 0, 2, 1)).reshape(BQ, n_dw * DW, VOCAB)
        out[q * BQ : (q + 1) * BQ] = full[:, :n_steps]
    return np.ascontiguousarray(out)


kernel.last_exec_time_ns = None
